# revision 18
# baseline (speedup 1.0000x reference)
"""Depth-gated 3x3 conv (DepthConv) Trainium2 Bass kernel, v2.

Problem: out[b,o,h,w] = sum_{c,kh,kw} x[b,c,h+kh-1,w+kw-1]
                        * exp(-|d[b,h,w] - d[b,h+kh-1,w+kw-1]|)
                        * weight[o,c,kh,kw]  + bias[o]
with B=8, Cin=Cout=64, H=W=128, zero padding.

Sharding: data-parallel over batch, one image per NeuronCore (8 cores).

v2 structure (vs v1):
  - center tap (k=4) has gate exp(0)=1: it skips gating entirely and its
    rhs streams straight from the band buffer into the main GEMM.
  - the other 8 taps form 4 chunk-pairs (1,2),(6,7) [+1 shift, buf A]
    and (0,3),(5,8) [+130 shift, buf B].
  - gates g=exp(-|d_tap-d_center|) computed bf16 in two column halves so
    the PE can start ~5us in; abs runs on DVE (stt max(x,-x)), exp on ACT.
  - pg broadcast via 4 concurrent ones-matmuls (row strips 0/32/64/96),
    consumed by a tuned ACT/DVE mix (ACT copies some psum tiles to SBUF
    bf16 for wide 2x DVE multiplies; one chunk multiplies from PSUM).
  - per-pair keep-warm dense matmul + early warmup keep the PE HAM clock
    gate at 8/8.
  - DMA triggers split across Sync (I/O) and GpSimd (gate relayout).
"""

import numpy as np

B, CIN, COUT, H, W = 8, 64, 64, 128, 128
HP, WP = H + 2, W + 2            # padded
NPAD = HP * WP                   # 16900
NXCOL = 16904                    # x staging buffer columns (padded + slack)
S = H * W                        # 16384 outputs per image
NB = 8                           # h-blocks
BH = H // NB                     # 16 rows per block
BLK = BH * W                     # 2048 outputs per block
TW = 512                         # psum tile width
BANDC = 2368                     # band buffer cols
BANDV = (BH + 1) * WP + W + 2    # 2340 band cols actually loaded
DHW = 1040                       # d72 half-window cols (needs (8-1)*130+128)
DCOL2 = 2176                     # d72 half-buffer cols: [tap 0:1040 | center 1088:2128]

# chunk j -> (low tap, high tap); pairs (1,2),(6,7) differ by +1 (buf A),
# (0,3),(5,8) by +130 (buf B). center tap 4 is ungated.
TAP = [(1, 2), (6, 7), (0, 3), (5, 8)]
CHUNK_BUF = ["A", "A", "B", "B"]
N_WARMUP = 16


def _split_multi_waits(nc, mybir):
    """Walrus encodes at most ONE sync wait per instruction; split Tile's
    multi-wait sync_info into single-wait NOPs on the same engine."""
    cnt = 0
    for f in nc.m.functions:
        for bb in f.blocks:
            newl = []
            for ins in bb.instructions:
                si = ins.sync_info
                if si is not None and si.on_wait and len(si.on_wait) > 1:
                    waits = list(si.on_wait)
                    for w in waits[:-1]:
                        cnt += 1
                        newl.append(
                            mybir.InstNoOp(
                                name=f"waitsplit-{cnt}",
                                ins=[],
                                outs=[],
                                engine=ins.engine,
                                sync_info=mybir.SyncInfo(on_wait=[w], on_update=[]),
                            )
                        )
                    ins.sync_info = mybir.SyncInfo(
                        on_wait=[waits[-1]], on_update=list(si.on_update)
                    )
                newl.append(ins)
            bb.instructions = newl
    return cnt


def build_nc():
    import os
    import concourse.bass as bass
    import concourse.mybir as mybir
    from concourse import tile

    dbg = os.environ.get("DC_DEBUG", "")

    f32 = mybir.dt.float32
    bf16 = mybir.dt.bfloat16
    Alu = mybir.AluOpType
    Act = mybir.ActivationFunctionType

    nc = bass.Bass()
    xa_d = nc.declare_dram_parameter("xa", [128, NXCOL], bf16, isOutput=False)
    xb_d = nc.declare_dram_parameter("xb", [128, NXCOL], bf16, isOutput=False)
    d72a_d = nc.declare_dram_parameter("d72a", [72, DCOL2], bf16, isOutput=False)
    d72b_d = nc.declare_dram_parameter("d72b", [72, DCOL2], bf16, isOutput=False)
    wt_d = nc.declare_dram_parameter("wt", [128, 320], bf16, isOutput=False)
    em_d = nc.declare_dram_parameter("em", [128, 512], bf16, isOutput=False)
    bias_d = nc.declare_dram_parameter("bias", [128], f32, isOutput=False)
    out_d = nc.declare_dram_parameter("out", [64, S], f32, isOutput=True)
    dbg_d = None
    if dbg == "dump":
        dbg_d = nc.declare_dram_parameter("dbg", [128, 8192], f32, isOutput=True)

    with tile.TileContext(nc) as tc:
        with (
            tc.tile_pool(name="consts", bufs=1) as consts,
            tc.tile_pool(name="gwork", bufs=2) as gwork,
            tc.tile_pool(name="bands", bufs=4) as bands,
            tc.tile_pool(name="imp", bufs=2) as imp,
            tc.tile_pool(name="gsp", bufs=2) as gsp,
            tc.tile_pool(name="outp", bufs=3) as outp,
            tc.tile_pool(name="pgp", bufs=7, space=bass.MemorySpace.PSUM) as pgp,
            tc.tile_pool(name="pop", bufs=1, space=bass.MemorySpace.PSUM) as pop,
        ):
            # ---- constants (small, first so warmup can start immediately) ----
            wt_sb = consts.tile([128, 320], bf16, tag="wt")
            nc.sync.dma_start(out=wt_sb[:], in_=wt_d[:])
            em_sb = consts.tile([128, 512], bf16, tag="em")
            nc.sync.dma_start(out=em_sb[:], in_=em_d[:])
            bias_sb = consts.tile([128, 1], f32, tag="bias")
            nc.sync.dma_start(out=bias_sb[:], in_=bias_d.rearrange("(p o) -> p o", o=1))
            d72h = []
            for hf, dd in enumerate((d72a_d, d72b_d)):
                t = consts.tile([72, DCOL2], bf16, tag=f"d72_{hf}")
                nc.sync.dma_start(out=t[:], in_=dd[:])
                d72h.append(t)

            # ---- PE warmup on weights (no gate dependency) ----
            wu = pgp.tile([128, TW], f32, tag="pg")
            for _ in range(N_WARMUP):
                nc.tensor.matmul(
                    wu[0:64, :],
                    wt_sb[0:128, 0:64],
                    em_sb[0:128, 0:TW],
                    start=True,
                    stop=True,
                    skip_group_check=True,
                )

            # ---- gates: per half h, g = exp(-|d_tap - d_center|) bf16 ----
            gexp = consts.tile([72, BLK], bf16, tag="gexp")
            for hf in range(2):
                gdel = gwork.tile([72, 1024], bf16, tag="gdel")
                gfin = gwork.tile([72, 1024], bf16, tag="gfin")
                win_t = d72h[hf][:, 0:DHW].rearrange("q (r w) -> q r w", w=WP)[
                    :, :8, :W
                ]
                win_c = d72h[hf][:, 1088:1088 + DHW].rearrange(
                    "q (r w) -> q r w", w=WP
                )[:, :8, :W]
                gdel_v = gdel[:].rearrange("q (r w) -> q r w", w=W)
                nc.vector.tensor_sub(gdel_v, win_t, win_c)
                nc.vector.scalar_tensor_tensor(
                    gfin[:], gdel[:], -1.0, gdel[:], Alu.mult, Alu.max
                )
                nc.scalar.activation(
                    gexp[:, 1024 * hf:1024 * (hf + 1)], gfin[:], Act.Exp, scale=-1.0
                )

            # ---- relayout gates to 4 row-group replicas per block ----
            # block 0 per half (ready earliest), blocks 1-7 full width.
            g9t = []
            for hb in range(NB):
                g9 = consts.tile([128, BLK], bf16, tag=f"g9_{hb}")
                g9t.append(g9)
            for hf in range(2):
                for r in range(4):
                    nc.gpsimd.dma_start(
                        out=g9t[0][32 * r:32 * r + 9, 1024 * hf:1024 * (hf + 1)],
                        in_=gexp[0:9, 1024 * hf:1024 * (hf + 1)],
                    )
            for hb in range(1, NB):
                for r in range(4):
                    nc.gpsimd.dma_start(
                        out=g9t[hb][32 * r:32 * r + 9, :],
                        in_=gexp[9 * hb:9 * hb + 9, :],
                    )

            # ---- main loop over h-blocks ----
            for hb in range(NB):
                banda = bands.tile([128, BANDC], bf16, tag="bandA")
                nc.sync.dma_start(
                    out=banda[:, :BANDV], in_=xa_d[:, 2080 * hb:2080 * hb + BANDV]
                )
                bandb = bands.tile([128, BANDC], bf16, tag="bandB")
                nc.sync.dma_start(
                    out=bandb[:, :BANDV], in_=xb_d[:, 2080 * hb:2080 * hb + BANDV]
                )
                g9 = g9t[hb][:]

                for q2 in range(2):
                    p = 2 * hb + q2          # pair index (1024 outputs)
                    # 1) gating ones-matmuls -> PSUM f32 [128,512] x 2 waves
                    pgs = [[None] * 4, [None] * 4]
                    for w in range(2):
                        qt = 2 * q2 + w
                        for j in range(4):
                            pg = pgp.tile([128, TW], f32, tag="pg")
                            nc.tensor.matmul(
                                pg[:, :],
                                em_sb[32 * j:32 * j + 9, 128 * j:128 * j + 128],
                                g9[32 * j:32 * j + 9, TW * qt:TW * (qt + 1)],
                                start=True,
                                stop=True,
                                tile_position=(32 * j, 0),
                            )
                            pgs[w][j] = pg

                    # 2) consume psum: chunks 0,1 ACT-merged; chunk 2 mixed
                    #    ACT+DVE copy; chunk 3 DVE-direct from PSUM.
                    gss = {}
                    for j in (0, 1, 2):
                        gs = gsp.tile([128, 2 * TW], bf16, tag=f"gs{j}")
                        if j == 2 and (p % 2 == 0):
                            nc.vector.tensor_copy(gs[:, 0:TW], pgs[0][j][:, :])
                            nc.vector.tensor_copy(gs[:, TW:2 * TW], pgs[1][j][:, :])
                        else:
                            nc.scalar.copy(gs[:, 0:TW], pgs[0][j][:, :])
                            nc.scalar.copy(gs[:, TW:2 * TW], pgs[1][j][:, :])
                        gss[j] = gs
                    # 3) gated im2col into bf16 SBUF
                    ims = []
                    for j in range(4):
                        kh, kw = divmod(TAP[j][0], 3)
                        band = banda if CHUNK_BUF[j] == "A" else bandb
                        im = imp.tile([128, 2 * TW], bf16, tag=f"im{j}")
                        if j == 3:
                            for w in range(2):
                                off = ((8 * q2 + 4 * w) + kh) * WP + kw
                                bw = band[0:128, off:off + 520].rearrange(
                                    "p (r w) -> p r w", w=WP
                                )[:, :4, :W]
                                pgv = pgs[w][j][0:128, :].rearrange(
                                    "p (r w) -> p r w", w=W
                                )
                                imv = im[:, TW * w:TW * (w + 1)].rearrange(
                                    "p (r w) -> p r w", w=W
                                )
                                nc.vector.tensor_tensor(imv, bw, pgv, Alu.mult)
                        else:
                            off = (8 * q2 + kh) * WP + kw
                            bw = band[0:128, off:off + 1040].rearrange(
                                "p (r w) -> p r w", w=WP
                            )[:, :8, :W]
                            gsv = gss[j][:].rearrange("p (r w) -> p r w", w=W)
                            imv = im[:].rearrange("p (r w) -> p r w", w=W)
                            nc.vector.tensor_tensor(imv, bw, gsv, Alu.mult)
                        ims.append(im)
                    # 4) main GEMM, col-paired halves
                    po = pop.tile([128, TW], f32, tag="po")
                    for j in range(4):
                        for h in range(2):
                            nc.tensor.matmul(
                                po[64 * h:64 * h + 64, :],
                                wt_sb[0:128, 64 * j:64 * j + 64],
                                ims[j][:, TW * h:TW * (h + 1)],
                                start=(j == 0),
                                stop=False,
                                tile_position=(0, 64 * h),
                                skip_group_check=True,
                            )
                    for h in range(2):
                        offc = ((8 * q2 + 4 * h) + 1) * WP + 1
                        cw = banda[0:64, offc:offc + 520].rearrange(
                            "p (r w) -> p r w", w=WP
                        )[:, :4, :W]
                        nc.tensor.matmul(
                            po[64 * h:64 * h + 64, :],
                            wt_sb[0:64, 256:320],
                            cw,
                            start=False,
                            stop=True,
                            tile_position=(0, 64 * h),
                            skip_group_check=True,
                        )
                    if dbg_d is not None and hb == 0 and q2 == 0:
                        dmp = outp.tile([128, 1024], f32, tag="dump")
                        nc.vector.tensor_copy(dmp[0:64, :], imc[:])
                        nc.sync.dma_start(out=dbg_d[:, 0:1024], in_=dmp[:])
                        dmp2 = outp.tile([128, 1024], f32, tag="dump2")
                        nc.vector.tensor_copy(dmp2[:], gss[0][:])
                        nc.sync.dma_start(out=dbg_d[:, 1024:2048], in_=dmp2[:])
                        dmp3 = outp.tile([128, 1024], f32, tag="dump3")
                        nc.vector.tensor_copy(dmp3[:], ims[0][:])
                        nc.sync.dma_start(out=dbg_d[:, 2048:3072], in_=dmp3[:])
                        dmp4 = outp.tile([128, 1024], f32, tag="dump4")
                        nc.vector.tensor_copy(dmp4[:], g9t[0][:, 0:1024])
                        nc.sync.dma_start(out=dbg_d[:, 3072:4096], in_=dmp4[:])
                        dmp5 = outp.tile([128, 1024], f32, tag="dump5")
                        nc.vector.tensor_copy(
                            dmp5[:], banda[:, 0:1024]
                        )
                        nc.sync.dma_start(out=dbg_d[:, 4096:5120], in_=dmp5[:])
                        dmp6 = outp.tile([128, 320], f32, tag="dump6")
                        nc.vector.tensor_copy(dmp6[:], wt_sb[:])
                        nc.sync.dma_start(out=dbg_d[:, 5120:5440], in_=dmp6[:])
                    # 5) bias add + single store for both 512-tiles
                    ot = outp.tile([128, TW], f32, tag="ot")
                    nc.scalar.activation(
                        ot[:], po[:], Act.Identity, bias=bias_sb[:], scale=1.0
                    )
                    if dbg_d is not None and hb == 0 and q2 == 0:
                        dmp7 = outp.tile([128, TW], f32, tag="dump7")
                        nc.vector.tensor_copy(dmp7[:], po[:])
                        nc.sync.dma_start(out=dbg_d[:, 5632:6144], in_=dmp7[:])
                        nc.sync.dma_start(out=dbg_d[:, 6144:6656], in_=ot[:])
                    nc.sync.dma_start(
                        out=out_d[:, 1024 * p:1024 * p + TW], in_=ot[0:64, :]
                    )
                    nc.sync.dma_start(
                        out=out_d[:, 1024 * p + TW:1024 * (p + 1)], in_=ot[64:128, :]
                    )
    _split_multi_waits(nc, mybir)
    return nc


# ---------------- host-side input layout prep ----------------

def _pad_flat(img):
    """[C,H,W] -> [C, NPAD] zero-padded flattened."""
    c = img.shape[0]
    p = np.zeros((c, HP, WP), np.float32)
    p[:, 1:1 + H, 1:1 + W] = img
    return p.reshape(c, NPAD)


def prep_x(x_b):
    """x_b [64,H,W] -> xa, xb [128, NXCOL] bf16: lower=padded x, upper
    shifted by +1 / +WP elements."""
    import ml_dtypes

    xp = _pad_flat(np.asarray(x_b, np.float32))
    base = np.zeros((CIN, NXCOL), np.float32)
    base[:, :NPAD] = xp
    upa = np.zeros_like(base)
    upa[:, :NXCOL - 1] = base[:, 1:]
    upb = np.zeros_like(base)
    upb[:, :NXCOL - WP] = base[:, WP:]
    bf = ml_dtypes.bfloat16
    return (
        np.concatenate([base, upa], 0).astype(bf),
        np.concatenate([base, upb], 0).astype(bf),
    )


def prep_d(depth_b):
    """depth_b [H,W] -> d72a, d72b [72, DCOL2] bf16 halves.
    Half h: cols 0:1040 = per-tap window rows 8h..8h+7, cols 1088:2128 =
    center window rows 8h..8h+7 (of each 16-row block)."""
    import ml_dtypes

    dp = _pad_flat(np.asarray(depth_b, np.float32)[None])[0]
    halves = []
    for hf in range(2):
        d = np.zeros((72, DCOL2), np.float32)
        for hb in range(NB):
            for k in range(9):
                kh, kw = divmod(k, 3)
                off = 2080 * hb + 1040 * hf + WP * kh + kw
                d[9 * hb + k, 0:DHW - 2] = dp[off:off + DHW - 2]
                offc = 2080 * hb + 1040 * hf + WP + 1
                d[9 * hb + k, 1088:1088 + DHW - 2] = dp[offc:offc + DHW - 2]
        halves.append(d.astype(ml_dtypes.bfloat16))
    return halves


def prep_w(weight):
    """weight [64,64,3,3] -> wt [128, 320] chunk-packed (4 tap-pair chunks
    + center), em [128, 512] ones-selector."""
    import ml_dtypes

    w2 = np.asarray(weight, np.float32).reshape(COUT, CIN, 9)
    wt = np.zeros((128, 320), np.float32)
    em = np.zeros((128, 512), np.float32)
    for j in range(4):
        for half in range(2):
            k = TAP[j][half]
            wt[64 * half:64 * half + 64, 64 * j:64 * j + 64] = w2[:, :, k].T
            em[32 * j + k, 128 * j + 64 * half:128 * j + 64 * half + 64] = 1.0
    wt[0:64, 256:320] = w2[:, :, 4].T
    bf = ml_dtypes.bfloat16
    return wt.astype(bf), em.astype(bf)


def make_in_maps(x, depth, weight, bias):
    wt, em = prep_w(weight)
    bias2 = np.ascontiguousarray(np.tile(np.asarray(bias, np.float32), 2))
    in_maps = []
    for b in range(B):
        xa, xb = prep_x(x[b])
        d72a, d72b = prep_d(np.asarray(depth)[b, 0])
        in_maps.append(
            {
                "xa": xa,
                "xb": xb,
                "d72a": d72a,
                "d72b": d72b,
                "wt": wt,
                "em": em,
                "bias": bias2,
            }
        )
    return in_maps


_NC = None


def run(x, depth, weight, bias, trace=False):
    global _NC
    from concourse.bass_utils import run_bass_kernel_spmd

    if _NC is None:
        _NC = build_nc()
    in_maps = make_in_maps(x, depth, weight, bias)
    res = run_bass_kernel_spmd(_NC, in_maps, list(range(B)), trace=trace)
    out = np.stack(
        [np.asarray(res.results[b]["out"]).reshape(COUT, H, W) for b in range(B)]
    )
    return out.astype(np.float32), res


def kernel(x, depth, weight, bias):
    out, _ = run(x, depth, weight, bias, trace=False)
    return out


# revision 19
# speedup vs baseline: 1.1920x; 1.1920x over previous
"""Depth-gated 3x3 conv (DepthConv) Trainium2 Bass kernel, v2.

Problem: out[b,o,h,w] = sum_{c,kh,kw} x[b,c,h+kh-1,w+kw-1]
                        * exp(-|d[b,h,w] - d[b,h+kh-1,w+kw-1]|)
                        * weight[o,c,kh,kw]  + bias[o]
with B=8, Cin=Cout=64, H=W=128, zero padding.

Sharding: data-parallel over batch, one image per NeuronCore (8 cores).

v2 structure (vs v1):
  - center tap (k=4) has gate exp(0)=1: it skips gating entirely and its
    rhs streams straight from the band buffer into the main GEMM.
  - the other 8 taps form 4 chunk-pairs (1,2),(6,7) [+1 shift, buf A]
    and (0,3),(5,8) [+130 shift, buf B].
  - gates g=exp(-|d_tap-d_center|) computed bf16 in two column halves so
    the PE can start ~5us in; abs runs on DVE (stt max(x,-x)), exp on ACT.
  - pg broadcast via 4 concurrent ones-matmuls (row strips 0/32/64/96),
    consumed by a tuned ACT/DVE mix (ACT copies some psum tiles to SBUF
    bf16 for wide 2x DVE multiplies; one chunk multiplies from PSUM).
  - per-pair keep-warm dense matmul + early warmup keep the PE HAM clock
    gate at 8/8.
  - DMA triggers split across Sync (I/O) and GpSimd (gate relayout).
"""

import numpy as np

B, CIN, COUT, H, W = 8, 64, 64, 128, 128
HP, WP = H + 2, W + 2            # padded
NPAD = HP * WP                   # 16900
NXCOL = 16904                    # x staging buffer columns (padded + slack)
S = H * W                        # 16384 outputs per image
NB = 8                           # h-blocks
BH = H // NB                     # 16 rows per block
BLK = BH * W                     # 2048 outputs per block
TW = 512                         # psum tile width
BANDC = 2368                     # band buffer cols
BANDV = (BH + 1) * WP + W + 2    # 2340 band cols actually loaded
DHW = 1040                       # d72 half-window cols (needs (8-1)*130+128)
DCOL2 = 2176                     # d72 half-buffer cols: [tap 0:1040 | center 1088:2128]

# chunk j -> (low tap, high tap); pairs (1,2),(6,7) differ by +1 (buf A),
# (0,3),(5,8) by +130 (buf B). center tap 4 is ungated.
TAP = [(1, 2), (6, 7), (0, 3), (5, 8)]
CHUNK_BUF = ["A", "A", "B", "B"]
N_WARMUP = 8


def _split_multi_waits(nc, mybir):
    """Walrus encodes at most ONE sync wait per instruction; split Tile's
    multi-wait sync_info into single-wait NOPs on the same engine."""
    cnt = 0
    for f in nc.m.functions:
        for bb in f.blocks:
            newl = []
            for ins in bb.instructions:
                si = ins.sync_info
                if si is not None and si.on_wait and len(si.on_wait) > 1:
                    waits = list(si.on_wait)
                    for w in waits[:-1]:
                        cnt += 1
                        newl.append(
                            mybir.InstNoOp(
                                name=f"waitsplit-{cnt}",
                                ins=[],
                                outs=[],
                                engine=ins.engine,
                                sync_info=mybir.SyncInfo(on_wait=[w], on_update=[]),
                            )
                        )
                    ins.sync_info = mybir.SyncInfo(
                        on_wait=[waits[-1]], on_update=list(si.on_update)
                    )
                newl.append(ins)
            bb.instructions = newl
    return cnt


def build_nc():
    import os
    import concourse.bass as bass
    import concourse.mybir as mybir
    from concourse import tile

    dbg = os.environ.get("DC_DEBUG", "")

    f32 = mybir.dt.float32
    bf16 = mybir.dt.bfloat16
    Alu = mybir.AluOpType
    Act = mybir.ActivationFunctionType

    nc = bass.Bass()
    xa_d = nc.declare_dram_parameter("xa", [128, NXCOL], bf16, isOutput=False)
    xb_d = nc.declare_dram_parameter("xb", [128, NXCOL], bf16, isOutput=False)
    d72a_d = nc.declare_dram_parameter("d72a", [72, DCOL2], bf16, isOutput=False)
    d72b_d = nc.declare_dram_parameter("d72b", [72, DCOL2], bf16, isOutput=False)
    wt_d = nc.declare_dram_parameter("wt", [128, 320], bf16, isOutput=False)
    em_d = nc.declare_dram_parameter("em", [128, 512], bf16, isOutput=False)
    bias_d = nc.declare_dram_parameter("bias", [128], f32, isOutput=False)
    out_d = nc.declare_dram_parameter("out", [64, S], f32, isOutput=True)
    dbg_d = None
    if dbg == "dump":
        dbg_d = nc.declare_dram_parameter("dbg", [128, 8192], f32, isOutput=True)

    with tile.TileContext(nc) as tc:
        with (
            tc.tile_pool(name="consts", bufs=1) as consts,
            tc.tile_pool(name="gwork", bufs=2) as gwork,
            tc.tile_pool(name="bands", bufs=4) as bands,
            tc.tile_pool(name="imp", bufs=2) as imp,
            tc.tile_pool(name="gsp", bufs=2) as gsp,
            tc.tile_pool(name="outp", bufs=3) as outp,
            tc.tile_pool(name="pgp", bufs=7, space=bass.MemorySpace.PSUM) as pgp,
            tc.tile_pool(name="pop", bufs=1, space=bass.MemorySpace.PSUM) as pop,
        ):
            # ---- constants (small, first so warmup can start immediately) ----
            wt_sb = consts.tile([128, 320], bf16, tag="wt")
            nc.sync.dma_start(out=wt_sb[:], in_=wt_d[:])
            em_sb = consts.tile([128, 512], bf16, tag="em")
            nc.sync.dma_start(out=em_sb[:], in_=em_d[:])
            bias_sb = consts.tile([128, 1], f32, tag="bias")
            nc.sync.dma_start(out=bias_sb[:], in_=bias_d.rearrange("(p o) -> p o", o=1))
            d72h = []
            for hf, dd in enumerate((d72a_d, d72b_d)):
                t = consts.tile([72, DCOL2], bf16, tag=f"d72_{hf}")
                nc.sync.dma_start(out=t[:], in_=dd[:])
                d72h.append(t)

            # ---- PE warmup on weights (no gate dependency) ----
            wu = pgp.tile([128, TW], f32, tag="pg")
            for _ in range(N_WARMUP):
                nc.tensor.matmul(
                    wu[0:64, :],
                    wt_sb[0:128, 0:64],
                    em_sb[0:128, 0:TW],
                    start=True,
                    stop=True,
                    skip_group_check=True,
                )

            # ---- gates: per half h, g = exp(-|d_tap - d_center|) bf16 ----
            gexp = consts.tile([72, BLK], bf16, tag="gexp")
            for hf in range(2):
                gdel = gwork.tile([72, 1024], bf16, tag="gdel")
                gfin = gwork.tile([72, 1024], bf16, tag="gfin")
                win_t = d72h[hf][:, 0:DHW].rearrange("q (r w) -> q r w", w=WP)[
                    :, :8, :W
                ]
                win_c = d72h[hf][:, 1088:1088 + DHW].rearrange(
                    "q (r w) -> q r w", w=WP
                )[:, :8, :W]
                gdel_v = gdel[:].rearrange("q (r w) -> q r w", w=W)
                nc.vector.tensor_sub(gdel_v, win_t, win_c)
                nc.vector.scalar_tensor_tensor(
                    gfin[:], gdel[:], -1.0, gdel[:], Alu.mult, Alu.max
                )
                nc.scalar.activation(
                    gexp[:, 1024 * hf:1024 * (hf + 1)], gfin[:], Act.Exp, scale=-1.0
                )

            # ---- relayout gates to 4 row-group replicas per block ----
            # block 0 per half (ready earliest), blocks 1-7 full width.
            g9t = []
            for hb in range(NB):
                g9 = consts.tile([128, BLK], bf16, tag=f"g9_{hb}")
                g9t.append(g9)
            for hf in range(2):
                for r in range(4):
                    nc.gpsimd.dma_start(
                        out=g9t[0][32 * r:32 * r + 9, 1024 * hf:1024 * (hf + 1)],
                        in_=gexp[0:9, 1024 * hf:1024 * (hf + 1)],
                    )
            for hb in range(1, NB):
                for r in range(4):
                    nc.gpsimd.dma_start(
                        out=g9t[hb][32 * r:32 * r + 9, :],
                        in_=gexp[9 * hb:9 * hb + 9, :],
                    )

            # ---- main loop over h-blocks ----
            for hb in range(NB):
                banda = bands.tile([128, BANDC], bf16, tag="bandA")
                nc.sync.dma_start(
                    out=banda[:, :BANDV], in_=xa_d[:, 2080 * hb:2080 * hb + BANDV]
                )
                bandb = bands.tile([128, BANDC], bf16, tag="bandB")
                nc.sync.dma_start(
                    out=bandb[:, :BANDV], in_=xb_d[:, 2080 * hb:2080 * hb + BANDV]
                )
                g9 = g9t[hb][:]

                for q2 in range(2):
                    p = 2 * hb + q2          # pair index (1024 outputs)
                    # 1) gating ones-matmuls -> PSUM f32 [128,512] x 2 waves
                    pgs = [[None] * 4, [None] * 4]
                    for w in range(2):
                        qt = 2 * q2 + w
                        for j in range(4):
                            pg = pgp.tile([128, TW], f32, tag="pg")
                            nc.tensor.matmul(
                                pg[:, :],
                                em_sb[32 * j:32 * j + 9, 128 * j:128 * j + 128],
                                g9[32 * j:32 * j + 9, TW * qt:TW * (qt + 1)],
                                start=True,
                                stop=True,
                                tile_position=(32 * j, 0),
                            )
                            pgs[w][j] = pg

                    # 2) consume psum: chunks 0,1 ACT-merged; chunk 2 mixed
                    #    ACT+DVE copy; chunk 3 DVE-direct from PSUM.
                    gss = {}
                    for j in (0, 1, 2):
                        gs = gsp.tile([128, 2 * TW], bf16, tag=f"gs{j}")
                        if j == 2 and (p % 2 == 0):
                            nc.vector.tensor_copy(gs[:, 0:TW], pgs[0][j][:, :])
                            nc.vector.tensor_copy(gs[:, TW:2 * TW], pgs[1][j][:, :])
                        else:
                            nc.scalar.copy(gs[:, 0:TW], pgs[0][j][:, :])
                            nc.scalar.copy(gs[:, TW:2 * TW], pgs[1][j][:, :])
                        gss[j] = gs
                    # 3) gated im2col into bf16 SBUF
                    ims = []
                    for j in range(4):
                        kh, kw = divmod(TAP[j][0], 3)
                        band = banda if CHUNK_BUF[j] == "A" else bandb
                        im = imp.tile([128, 2 * TW], bf16, tag=f"im{j}")
                        if j == 3:
                            for w in range(2):
                                off = ((8 * q2 + 4 * w) + kh) * WP + kw
                                bw = band[0:128, off:off + 520].rearrange(
                                    "p (r w) -> p r w", w=WP
                                )[:, :4, :W]
                                pgv = pgs[w][j][0:128, :].rearrange(
                                    "p (r w) -> p r w", w=W
                                )
                                imv = im[:, TW * w:TW * (w + 1)].rearrange(
                                    "p (r w) -> p r w", w=W
                                )
                                nc.vector.tensor_tensor(imv, bw, pgv, Alu.mult)
                        else:
                            off = (8 * q2 + kh) * WP + kw
                            bw = band[0:128, off:off + 1040].rearrange(
                                "p (r w) -> p r w", w=WP
                            )[:, :8, :W]
                            gsv = gss[j][:].rearrange("p (r w) -> p r w", w=W)
                            imv = im[:].rearrange("p (r w) -> p r w", w=W)
                            nc.vector.tensor_tensor(imv, bw, gsv, Alu.mult)
                        ims.append(im)
                    # 4) main GEMM, col-paired halves
                    po = pop.tile([128, TW], f32, tag="po")
                    for j in range(4):
                        for h in range(2):
                            nc.tensor.matmul(
                                po[64 * h:64 * h + 64, :],
                                wt_sb[0:128, 64 * j:64 * j + 64],
                                ims[j][:, TW * h:TW * (h + 1)],
                                start=(j == 0),
                                stop=False,
                                tile_position=(0, 64 * h),
                                skip_group_check=True,
                            )
                    for h in range(2):
                        offc = ((8 * q2 + 4 * h) + 1) * WP + 1
                        cw = banda[0:64, offc:offc + 520].rearrange(
                            "p (r w) -> p r w", w=WP
                        )[:, :4, :W]
                        nc.tensor.matmul(
                            po[64 * h:64 * h + 64, :],
                            wt_sb[0:64, 256:320],
                            cw,
                            start=False,
                            stop=True,
                            tile_position=(0, 64 * h),
                            skip_group_check=True,
                        )
                    if dbg_d is not None and hb == 0 and q2 == 0:
                        dmp = outp.tile([128, 1024], f32, tag="dump")
                        nc.vector.tensor_copy(dmp[0:64, :], imc[:])
                        nc.sync.dma_start(out=dbg_d[:, 0:1024], in_=dmp[:])
                        dmp2 = outp.tile([128, 1024], f32, tag="dump2")
                        nc.vector.tensor_copy(dmp2[:], gss[0][:])
                        nc.sync.dma_start(out=dbg_d[:, 1024:2048], in_=dmp2[:])
                        dmp3 = outp.tile([128, 1024], f32, tag="dump3")
                        nc.vector.tensor_copy(dmp3[:], ims[0][:])
                        nc.sync.dma_start(out=dbg_d[:, 2048:3072], in_=dmp3[:])
                        dmp4 = outp.tile([128, 1024], f32, tag="dump4")
                        nc.vector.tensor_copy(dmp4[:], g9t[0][:, 0:1024])
                        nc.sync.dma_start(out=dbg_d[:, 3072:4096], in_=dmp4[:])
                        dmp5 = outp.tile([128, 1024], f32, tag="dump5")
                        nc.vector.tensor_copy(
                            dmp5[:], banda[:, 0:1024]
                        )
                        nc.sync.dma_start(out=dbg_d[:, 4096:5120], in_=dmp5[:])
                        dmp6 = outp.tile([128, 320], f32, tag="dump6")
                        nc.vector.tensor_copy(dmp6[:], wt_sb[:])
                        nc.sync.dma_start(out=dbg_d[:, 5120:5440], in_=dmp6[:])
                    # 5) bias add + single store for both 512-tiles
                    ot = outp.tile([128, TW], f32, tag="ot")
                    nc.scalar.activation(
                        ot[:], po[:], Act.Identity, bias=bias_sb[:], scale=1.0
                    )
                    if dbg_d is not None and hb == 0 and q2 == 0:
                        dmp7 = outp.tile([128, TW], f32, tag="dump7")
                        nc.vector.tensor_copy(dmp7[:], po[:])
                        nc.sync.dma_start(out=dbg_d[:, 5632:6144], in_=dmp7[:])
                        nc.sync.dma_start(out=dbg_d[:, 6144:6656], in_=ot[:])
                    nc.sync.dma_start(
                        out=out_d[:, 1024 * p:1024 * p + TW], in_=ot[0:64, :]
                    )
                    nc.sync.dma_start(
                        out=out_d[:, 1024 * p + TW:1024 * (p + 1)], in_=ot[64:128, :]
                    )
    _split_multi_waits(nc, mybir)
    return nc


# ---------------- host-side input layout prep ----------------

def _pad_flat(img):
    """[C,H,W] -> [C, NPAD] zero-padded flattened."""
    c = img.shape[0]
    p = np.zeros((c, HP, WP), np.float32)
    p[:, 1:1 + H, 1:1 + W] = img
    return p.reshape(c, NPAD)


def prep_x(x_b):
    """x_b [64,H,W] -> xa, xb [128, NXCOL] bf16: lower=padded x, upper
    shifted by +1 / +WP elements."""
    import ml_dtypes

    xp = _pad_flat(np.asarray(x_b, np.float32))
    base = np.zeros((CIN, NXCOL), np.float32)
    base[:, :NPAD] = xp
    upa = np.zeros_like(base)
    upa[:, :NXCOL - 1] = base[:, 1:]
    upb = np.zeros_like(base)
    upb[:, :NXCOL - WP] = base[:, WP:]
    bf = ml_dtypes.bfloat16
    return (
        np.concatenate([base, upa], 0).astype(bf),
        np.concatenate([base, upb], 0).astype(bf),
    )


def prep_d(depth_b):
    """depth_b [H,W] -> d72a, d72b [72, DCOL2] bf16 halves.
    Half h: cols 0:1040 = per-tap window rows 8h..8h+7, cols 1088:2128 =
    center window rows 8h..8h+7 (of each 16-row block)."""
    import ml_dtypes

    dp = _pad_flat(np.asarray(depth_b, np.float32)[None])[0]
    halves = []
    for hf in range(2):
        d = np.zeros((72, DCOL2), np.float32)
        for hb in range(NB):
            for k in range(9):
                kh, kw = divmod(k, 3)
                off = 2080 * hb + 1040 * hf + WP * kh + kw
                d[9 * hb + k, 0:DHW - 2] = dp[off:off + DHW - 2]
                offc = 2080 * hb + 1040 * hf + WP + 1
                d[9 * hb + k, 1088:1088 + DHW - 2] = dp[offc:offc + DHW - 2]
        halves.append(d.astype(ml_dtypes.bfloat16))
    return halves


def prep_w(weight):
    """weight [64,64,3,3] -> wt [128, 320] chunk-packed (4 tap-pair chunks
    + center), em [128, 512] ones-selector."""
    import ml_dtypes

    w2 = np.asarray(weight, np.float32).reshape(COUT, CIN, 9)
    wt = np.zeros((128, 320), np.float32)
    em = np.zeros((128, 512), np.float32)
    for j in range(4):
        for half in range(2):
            k = TAP[j][half]
            wt[64 * half:64 * half + 64, 64 * j:64 * j + 64] = w2[:, :, k].T
            em[32 * j + k, 128 * j + 64 * half:128 * j + 64 * half + 64] = 1.0
    wt[0:64, 256:320] = w2[:, :, 4].T
    bf = ml_dtypes.bfloat16
    return wt.astype(bf), em.astype(bf)


def make_in_maps(x, depth, weight, bias):
    wt, em = prep_w(weight)
    bias2 = np.ascontiguousarray(np.tile(np.asarray(bias, np.float32), 2))
    in_maps = []
    for b in range(B):
        xa, xb = prep_x(x[b])
        d72a, d72b = prep_d(np.asarray(depth)[b, 0])
        in_maps.append(
            {
                "xa": xa,
                "xb": xb,
                "d72a": d72a,
                "d72b": d72b,
                "wt": wt,
                "em": em,
                "bias": bias2,
            }
        )
    return in_maps


_NC = None


def run(x, depth, weight, bias, trace=False):
    global _NC
    from concourse.bass_utils import run_bass_kernel_spmd

    if _NC is None:
        _NC = build_nc()
    in_maps = make_in_maps(x, depth, weight, bias)
    res = run_bass_kernel_spmd(_NC, in_maps, list(range(B)), trace=trace)
    out = np.stack(
        [np.asarray(res.results[b]["out"]).reshape(COUT, H, W) for b in range(B)]
    )
    return out.astype(np.float32), res


def kernel(x, depth, weight, bias):
    out, _ = run(x, depth, weight, bias, trace=False)
    return out
